# revision 42
# baseline (speedup 1.0000x reference)
"""AttentionBlock (GroupNorm -> QKV 1x1 -> single-head attention -> out proj -> residual)
for x:(4,512,64,64) f32, distributed over 8 NeuronCores.

Sharding: data-parallel over batch, 2 cores per sample, each core owns 2048 of
the 4096 query positions. Each core receives a column-ROTATED copy of its
sample (local 2048 positions first) so the compiled program is identical on
every core (SPMD).

Numerics/engine strategy (fp8e4m3 + DoubleRow for all heavy matmuls; fits the
rel-err budget with ~4x margin per a host-side quantization study):
  1) stats stream: x chunks HBM->SBUF f32; DVE bn_stats; ACT casts x into an
     fp8 SBUF-resident copy. Cross-partition group reduction via PE transposes
     yields the per-channel GroupNorm affine (a, b): xn = a*x + b.
  2) weights: W' = W diag(a) applied during the f32 -> fp8 cast (per-partition
     scale). Biases fold entirely into per-channel vectors: b'_q = bq + Wq b
     (tiny N=1 f32 matmuls, channel layout); the K bias is DROPPED (it adds a
     per-query constant to logits, exactly cancelled by softmax); the V bias
     folds THROUGH the output projection: bo' = bo + Wo (bv + Wv b), so the
     V/O path never adds a bias in the hot loop.
  3) Q/K/V projections: fp8 DoubleRow matmuls (256-deep contraction per
     instruction, 0.5 cyc/row); PSUM -> SBUF casts to fp8 rotate across
     DVE/ACT (GPSIMD cannot read PSUM; it handles SBUF-side adds + DMA
     issue only). Score matmuls are software-pipelined ahead of the previous
     block's O matmuls so the in-order PE queue never blocks ACT.
  4) Attention per 512-query block over 256-key blocks: scores K^T Q fp8-DR
     into a 2-bank PSUM tile; ACT exp (scale, constant max-shift -4, softmax-
     invariant, keeps E in fp8 range) emits E fp8; O += V^T E fp8-DR; the
     softmax denominator accumulates in PSUM via an fp8 ones-DR matmul
     (no vector-engine reduction). PSUM: 4 (O) + 2 (scores) + 1 (denom) +
     1 (bcast/out-proj) = 8 banks.
  5) late normalization: O is drained to fp8 unnormalized (pre-scaled 1/128),
     freeing its PSUM banks immediately; out = (Wo O) * (128/denom) + bo' + x
     with the residual taken from the SBUF-resident f32 local half of x. The
     output tail's PE work is interleaved into the next query block's loop.
"""

import sys

sys.path.insert(0, "/opt/trn_rl_repo")

import numpy as np
from contextlib import ExitStack

import concourse.bass as bass
import concourse.tile as tile
from concourse import bacc, mybir
from concourse.masks import make_identity

F32 = mybir.dt.float32
F32R = mybir.dt.float32r
FP8 = mybir.dt.float8e4
DR = mybir.MatmulPerfMode.DoubleRow
Exp = mybir.ActivationFunctionType.Exp
Identity = mybir.ActivationFunctionType.Identity

C = 512          # channels
HW = 4096        # spatial positions per sample
L = 2048         # query positions per core
P = 128          # partitions
CO = C // P      # 4 channel chunks
NG = 32          # groups
GS = C // NG     # 16 channels per group
G_PER_CO = P // GS  # 8 groups per 128-partition chunk
EPS = 1e-6
SCALE = C ** -0.5
MSHIFT = 4.0     # constant subtracted inside exp (softmax-invariant)
IB = 512         # query block
NIB = L // IB    # 4
JB = 256         # key block per attention step
NJB = HW // JB   # 16
NXC = HW // 512  # 8 x-stream chunks
B = 4            # batch
NCORES = 8

_cached = {}


def build_program(reps: int = 1):
    nc = bacc.Bacc(None, target_bir_lowering=False)

    xf = nc.declare_dram_parameter("xf", [C, HW], F32, isOutput=False)
    wqt_d = nc.declare_dram_parameter("wqt", [C, C], F32, isOutput=False)
    wkt_d = nc.declare_dram_parameter("wkt", [C, C], F32, isOutput=False)
    wvt_d = nc.declare_dram_parameter("wvt", [C, C], F32, isOutput=False)
    wot_d = nc.declare_dram_parameter("wot", [C, C], F32, isOutput=False)
    bq_d = nc.declare_dram_parameter("bq", [C], F32, isOutput=False)
    bk_d = nc.declare_dram_parameter("bk", [C], F32, isOutput=False)
    bv_d = nc.declare_dram_parameter("bv", [C], F32, isOutput=False)
    bo_d = nc.declare_dram_parameter("bo", [C], F32, isOutput=False)
    gamma_d = nc.declare_dram_parameter("gamma", [C], F32, isOutput=False)
    beta_d = nc.declare_dram_parameter("beta", [C], F32, isOutput=False)
    y = nc.declare_dram_parameter("y", [C, L], F32, isOutput=True)

    # [c, j] -> [cp, coo, j] with c = coo*128 + cp
    xf_t = xf[:].rearrange("(coo cp) j -> cp coo j", cp=P)
    y_t = y[:].rearrange("(coo cp) i -> cp coo i", cp=P)

    with tile.TileContext(nc) as tc:
        for _rep in range(reps):
          with ExitStack() as ctx:
            consts = ctx.enter_context(tc.tile_pool(name="consts", bufs=1))
            big = ctx.enter_context(tc.tile_pool(name="big", bufs=1))
            po = ctx.enter_context(tc.psum_pool(name="po", bufs=4))

            ident = consts.tile([P, P], F32)
            make_identity(nc, ident)
            onesf = consts.tile([P, 2, 16], F32)
            nc.vector.memset(onesf, 1.0)
            ones8 = consts.tile([P, 2, 16], FP8)
            nc.vector.tensor_copy(out=ones8, in_=onesf)
            ones_rowf = consts.tile([1, P], F32)
            nc.vector.memset(ones_rowf, 1.0)
            ones_row = consts.tile([1, P], F32R)
            nc.vector.tensor_copy(out=ones_row, in_=ones_rowf)
            mshift = consts.tile([P, 1], F32)
            nc.vector.memset(mshift, -MSHIFT)
            zero_b = consts.tile([P, 1], F32)
            nc.vector.memset(zero_b, 0.0)
            eps_t = consts.tile([CO, 1], F32)
            nc.vector.memset(eps_t, EPS)

            def load_chan_vec(name, dsrc):
                t = consts.tile([P, CO], F32, tag=name)
                # gpsimd (SWDGE) issue: keeps SP free for the x stream
                nc.gpsimd.dma_start(
                    out=t, in_=dsrc[:].rearrange("(coo cp) -> cp coo", cp=P)
                )
                return t

            gamma_sb = load_chan_vec("gamma_sb", gamma_d)
            beta_sb = load_chan_vec("beta_sb", beta_d)
            bo_ch = load_chan_vec("bo_ch", bo_d)
            bq_ch = load_chan_vec("bq_ch", bq_d)
            bv_ch = load_chan_vec("bv_ch", bv_d)

            # fp8 operand stores (SBUF-resident); xloc keeps the local half
            # of x in f32 for the residual add (saves a 4MB re-stream)
            x8 = big.tile([P, CO, HW], FP8, tag="x8")
            xloc = big.tile([P, CO, L], F32, tag="xloc")
            K8 = big.tile([P, CO, HW], FP8, tag="K8")
            VT8 = big.tile([P, NJB, 2, C], FP8, tag="VT8")
            Q8 = big.tile([P, CO, L], FP8, tag="Q8")
            Wq8 = big.tile([P, CO, C], FP8, tag="Wq8")
            Wk8 = big.tile([P, CO, C], FP8, tag="Wk8")
            Wv8 = big.tile([P, CO, C], FP8, tag="Wv8")
            Wo8 = big.tile([P, CO, C], FP8, tag="Wo8")
            bqf = consts.tile([P, CO], F32, tag="bqf")
            bvf = consts.tile([P, CO], F32, tag="bvf")
            bof = consts.tile([P, CO], F32, tag="bof")

            with ExitStack() as phctx:
                phW = phctx.enter_context(tc.tile_pool(name="phW", bufs=1))
                ph1 = phctx.enter_context(tc.tile_pool(name="ph1", bufs=1))
                xchunk = phctx.enter_context(tc.tile_pool(name="xchunk",
                                                          bufs=2))
                pp1 = phctx.enter_context(tc.psum_pool(name="pp1", bufs=2))

                WqT = phW.tile([P, CO, C], F32, tag="WqT")
                WkT = phW.tile([P, CO, C], F32, tag="WkT")
                WvT = phW.tile([P, CO, C], F32, tag="WvT")
                WoT = phW.tile([P, CO, C], F32, tag="WoT")
                for WT, wsrc in (
                    (WqT, wqt_d), (WkT, wkt_d), (WvT, wvt_d), (WoT, wot_d)
                ):
                    # ACT-queue issue: SP stays dedicated to the x stream
                    nc.scalar.dma_start(
                        out=WT,
                        in_=wsrc[:].rearrange("(cio cp) co -> cp cio co",
                                              cp=P),
                    )
                # ---------- Phase A: stats stream + x -> fp8 ----------
                # x arrives in 4 DMAs of 1024 positions; the first two land in
                # the resident xloc (local query half, reused as the residual)
                stats = ph1.tile([P, CO, NXC, 6], F32, tag="stats")
                for ci in range(4):
                    csl = slice(ci * 1024, (ci + 1) * 1024)
                    if ci < 2:
                        xc = xloc[:, :, ci * 1024 : (ci + 1) * 1024]
                    else:
                        xc = xchunk.tile([P, CO, 1024], F32, tag="xc",
                                         name=f"xcs{ci}")
                    nc.sync.dma_start(out=xc, in_=xf_t[:, :, csl])
                    for coo in range(CO):
                        for hf in range(2):
                            nc.vector.bn_stats(
                                out=stats[:, coo, 2 * ci + hf, :],
                                in_=xc[:, coo, hf * 512 : (hf + 1) * 512],
                            )
                    nc.gpsimd.tensor_copy(out=x8[:, :, csl], in_=xc)
                mv = ph1.tile([P, CO, 2], F32, tag="mv")
                for coo in range(CO):
                    nc.vector.bn_aggr(out=mv[:, coo, :],
                                      in_=stats[:, coo, :, :])
                # T_in cols 0:4 per-channel mean, 4:8 per-channel E[x^2]
                T_in = ph1.tile([P, 8], F32, tag="T_in")
                nc.vector.tensor_copy(T_in[:, 0:CO], mv[:, :, 0])
                nc.vector.tensor_tensor(
                    out=T_in[:, CO : 2 * CO], in0=mv[:, :, 0], in1=mv[:, :, 0],
                    op=mybir.AluOpType.mult,
                )
                nc.vector.tensor_tensor(
                    out=T_in[:, CO : 2 * CO], in0=T_in[:, CO : 2 * CO],
                    in1=mv[:, :, 1], op=mybir.AluOpType.add,
                )
                tps = pp1.tile([8, P], F32, tag="wtp")
                nc.tensor.transpose(tps, T_in, ident)
                T_sb = ph1.tile([8, P], F32, tag="T_sb")
                nc.vector.tensor_copy(T_sb, tps)
                G = ph1.tile([8, G_PER_CO], F32, tag="G")
                nc.vector.reduce_sum(
                    out=G, in_=T_sb.rearrange("p (g s) -> p g s", s=GS),
                    axis=mybir.AxisListType.X,
                )
                G2 = ph1.tile([CO, G_PER_CO], F32, tag="G2")
                nc.sync.dma_start(out=G2, in_=G[CO : 2 * CO, :])
                mean_g = ph1.tile([CO, G_PER_CO], F32, tag="mean_g")
                nc.scalar.mul(out=mean_g, in_=G[0:CO, :], mul=1.0 / GS)
                var_g = ph1.tile([CO, G_PER_CO], F32, tag="var_g")
                nc.vector.tensor_tensor(
                    out=var_g, in0=mean_g, in1=mean_g, op=mybir.AluOpType.mult
                )
                nc.vector.tensor_scalar(
                    out=G2, in0=G2, scalar1=1.0 / GS, scalar2=None,
                    op0=mybir.AluOpType.mult,
                )
                nc.vector.tensor_tensor(
                    out=var_g, in0=G2, in1=var_g, op=mybir.AluOpType.subtract
                )
                # rstd = exp(-0.5 ln(var+eps)): keeps every ACT func in one
                # table set (ln/exp/identity) so no table reloads mid-kernel
                rstd_g = ph1.tile([CO, G_PER_CO], F32, tag="rstd_g")
                nc.scalar.activation(
                    out=rstd_g, in_=var_g,
                    func=mybir.ActivationFunctionType.Ln,
                    bias=eps_t, scale=1.0,
                )
                nc.scalar.activation(
                    out=rstd_g, in_=rstd_g,
                    func=Exp, scale=-0.5, bias=zero_b[0:CO, :],
                )

                # group -> channel broadcast along free, then PE transpose
                Bm = ph1.tile([CO, P], F32, tag="Bm")
                Br = ph1.tile([CO, P], F32, tag="Br")
                for src, dst in ((mean_g, Bm), (rstd_g, Br)):
                    bc = bass.AP(
                        tensor=src.tensor, offset=src.offset,
                        ap=[src.ap[0], src.ap[1], [0, GS]],
                    )
                    nc.vector.tensor_copy(
                        dst.rearrange("p (g s) -> p g s", s=GS), bc
                    )
                mean_ps = pp1.tile([P, CO], F32, tag="wtp", name="mean_ps")
                rstd_ps = pp1.tile([P, CO], F32, tag="wtp", name="rstd_ps")
                nc.tensor.transpose(mean_ps, Bm, ident[0:CO, 0:CO])
                nc.tensor.transpose(rstd_ps, Br, ident[0:CO, 0:CO])
                # a = gamma * rstd ; b = beta - mean * a   [128, 4] channel
                a_ch = consts.tile([P, CO], F32, tag="a_ch")
                b_ch = consts.tile([P, CO], F32, tag="b_ch")
                nc.vector.tensor_tensor(
                    out=a_ch, in0=gamma_sb, in1=rstd_ps, op=mybir.AluOpType.mult
                )
                nc.vector.tensor_tensor(
                    out=b_ch, in0=mean_ps, in1=a_ch, op=mybir.AluOpType.mult
                )
                nc.vector.tensor_tensor(
                    out=b_ch, in0=beta_sb, in1=b_ch, op=mybir.AluOpType.subtract
                )

                # ---------- Phase B: bias folds + weight scale/cast ----------
                # b'_w = b_w + W b_ch in channel layout via N=1 f32 matmuls
                def fold_bias(WT, rhs_col_src, base_vec, bdst, nm):
                    for coo in range(CO):
                        pb = pp1.tile([P, 1], F32, tag="pb",
                                      name=f"pb_{nm}{coo}")
                        for cio in range(CO):
                            nc.tensor.matmul(
                                pb,
                                lhsT=WT[:, cio, coo * P : (coo + 1) * P],
                                rhs=rhs_col_src[:, cio : cio + 1],
                                start=(cio == 0), stop=(cio == CO - 1),
                            )
                        nc.vector.tensor_tensor(
                            out=bdst[:, coo : coo + 1], in0=pb,
                            in1=base_vec[:, coo : coo + 1],
                            op=mybir.AluOpType.add,
                        )

                fold_bias(WqT, b_ch, bq_ch, bqf, "q")
                # K bias dropped: it only adds a per-query constant to the
                # logits, which softmax over keys cancels exactly
                fold_bias(WvT, b_ch, bv_ch, bvf, "v")
                # V bias folds through the output projection:
                # bo' = bo + Wo b'v
                fold_bias(WoT, bvf, bo_ch, bof, "o")

                # W'8 = fp8(W * a) — per-input-channel scale; split engines
                for cio in range(CO):
                    nc.vector.tensor_scalar_mul(
                        Wq8[:, cio, :], WqT[:, cio, :], a_ch[:, cio : cio + 1]
                    )
                    nc.scalar.activation(
                        out=Wk8[:, cio, :], in_=WkT[:, cio, :], func=Identity,
                        scale=a_ch[:, cio : cio + 1], bias=zero_b,
                    )
                    nc.gpsimd.tensor_scalar_mul(
                        Wv8[:, cio, :], WvT[:, cio, :], a_ch[:, cio : cio + 1]
                    )
                nc.vector.tensor_copy(out=Wo8, in_=WoT)

                # PSUM->SBUF fp8 cast with per-partition bias, engine-rotated
                # GPSIMD cannot read PSUM (walrus birverifier) — PSUM->SBUF
                # casts run on DVE/ACT only
                def cast_bias(dst, src, bias_ap, eng):
                    if eng == 0:
                        nc.vector.tensor_scalar(
                            out=dst, in0=src, scalar1=bias_ap, scalar2=None,
                            op0=mybir.AluOpType.add,
                        )
                    else:
                        nc.scalar.activation(
                            out=dst, in_=src, func=Identity, bias=bias_ap,
                            scale=1.0,
                        )

                # ---------- Phase C: Q projection (fp8 DR) ----------
                nce = 0
                for coo in range(CO):
                    for ib in range(NIB):
                        isl = slice(ib * IB, (ib + 1) * IB)
                        pq = po.tile([P, IB], F32, tag="ops",
                                     name=f"pq{ib}_{coo}")
                        for q in range(2):
                            nc.tensor.matmul(
                                pq,
                                lhsT=Wq8[:, 2 * q : 2 * q + 2,
                                         coo * P : (coo + 1) * P],
                                rhs=x8[:, 2 * q : 2 * q + 2, isl],
                                start=(q == 0), stop=(q == 1), perf_mode=DR,
                            )
                        cast_bias(Q8[:, coo, isl], pq,
                                  bqf[:, coo : coo + 1], nce % 2)
                        nce += 1

                # ---------- Phase D: K and V^T, interleaved per 512-col
                # block so attention consumption order matches production ----
                for s in range(NXC):
                    jsl = slice(s * 512, (s + 1) * 512)
                    for coo in range(CO):
                        pk = po.tile([P, 512], F32, tag="ops",
                                     name=f"pk{s}_{coo}")
                        for q in range(2):
                            nc.tensor.matmul(
                                pk,
                                lhsT=Wk8[:, 2 * q : 2 * q + 2,
                                         coo * P : (coo + 1) * P],
                                rhs=x8[:, 2 * q : 2 * q + 2, jsl],
                                start=(q == 0), stop=(q == 1), perf_mode=DR,
                            )
                        # no bias (dropped: softmax-invariant): pure cast
                        dstk = K8[:, coo, jsl]
                        eng = nce % 2
                        nce += 1
                        if eng == 0:
                            nc.vector.tensor_copy(out=dstk, in_=pk)
                        else:
                            nc.scalar.activation(
                                out=dstk, in_=pk, func=Identity, scale=1.0,
                                bias=zero_b,
                            )
                    for jb in (2 * s, 2 * s + 1):
                        for jh in range(2):
                            jpos = jb * JB + jh * P
                            pv = po.tile([P, C], F32, tag="ops",
                                         name=f"pv{jb}_{jh}")
                            for q in range(2):
                                nc.tensor.matmul(
                                    pv,
                                    lhsT=x8[:, 2 * q : 2 * q + 2,
                                            jpos : jpos + P],
                                    rhs=Wv8[:, 2 * q : 2 * q + 2, :],
                                    start=(q == 0), stop=(q == 1),
                                    perf_mode=DR,
                                )
                            dstv = VT8[:, jb, jh, :]
                            eng = nce % 2
                            nce += 1
                            if eng == 0:
                                nc.vector.tensor_copy(out=dstv, in_=pv)
                            else:
                                nc.scalar.activation(
                                    out=dstv, in_=pv, func=Identity,
                                    scale=1.0, bias=zero_b,
                                )

            # ---------- Phase E: attention + output ----------
            # Late normalization: O is cast to fp8 unnormalized (pre-scaled
            # by 1/128 to stay in fp8 range) the moment the j-loop ends,
            # releasing the 4 O PSUM banks immediately; the output projection
            # runs on unnormalized O and the 128/denom factor is applied in
            # the final DVE op. The out-projection PE work is interleaved
            # into the NEXT query block's pipeline slots so the in-order PE
            # queue never stalls on the output tail.
            OSC = 1.0 / 128.0
            with (
                tc.tile_pool(name="att", bufs=2) as att,
                tc.tile_pool(name="esb", bufs=4) as esb,
                tc.psum_pool(name="pspt", bufs=2) as pspt,
                tc.psum_pool(name="pdns", bufs=1) as pdns,
                tc.psum_pool(name="pfb", bufs=1) as pfb,
            ):
                tail = []  # pending PE op-groups from the previous ib

                def gp_bias_res_add(t, coo, xres):
                    # t += bo' (free-broadcast AP) ; t += x  — SBUF-only ops
                    # on the otherwise idle GPSIMD engine
                    bof_b = bass.AP(
                        tensor=bof.tensor, offset=bof.offset + coo,
                        ap=[bof.ap[0], [0, IB]],
                    )
                    nc.gpsimd.tensor_tensor(
                        out=t, in0=t, in1=bof_b, op=mybir.AluOpType.add,
                    )
                    nc.gpsimd.tensor_tensor(
                        out=t, in0=t, in1=xres[:, coo, :],
                        op=mybir.AluOpType.add,
                    )


                def emit_out_tail(ib, O8, bcast_sb, ysb, xres, isl):
                    def do_bcast(recip=None, _ib=ib):
                        bcast_ps = pfb.tile([P, IB], F32, tag="fps",
                                            name=f"bc{_ib}")
                        nc.tensor.matmul(
                            bcast_ps, lhsT=ones_row, rhs=recip, start=True,
                            stop=True,
                        )
                        # 128/denom: undo the 1/128 O pre-scale
                        nc.vector.tensor_scalar(
                            out=bcast_sb, in0=bcast_ps, scalar1=128.0,
                            scalar2=None, op0=mybir.AluOpType.mult,
                        )

                    def do_coo(coo, _ib=ib):
                        fps = pfb.tile([P, IB], F32, tag="fps",
                                       name=f"fps{_ib}_{coo}")
                        for q in range(2):
                            nc.tensor.matmul(
                                fps,
                                lhsT=Wo8[:, 2 * q : 2 * q + 2,
                                         coo * P : (coo + 1) * P],
                                rhs=O8[:, 2 * q : 2 * q + 2, :],
                                start=(q == 0), stop=(q == 1), perf_mode=DR,
                            )
                        t = ysb[:, coo, :]
                        nc.vector.tensor_tensor(
                            out=t, in0=fps, in1=bcast_sb,
                            op=mybir.AluOpType.mult,
                        )
                        if coo % 2 == 0:
                            nc.vector.scalar_tensor_tensor(
                                out=t, in0=t, scalar=bof[:, coo : coo + 1],
                                in1=xres[:, coo, :],
                                op0=mybir.AluOpType.add,
                                op1=mybir.AluOpType.add,
                            )
                        else:
                            nc.gpsimd.scalar_tensor_tensor(
                                out=t, in0=t, scalar=bof[:, coo : coo + 1],
                                in1=xres[:, coo, :],
                                op0=mybir.AluOpType.add,
                                op1=mybir.AluOpType.add,
                            )

                    def do_store(_isl=isl, _ysb=ysb):
                        nc.sync.dma_start(out=y_t[:, :, _isl], in_=_ysb)

                    return [do_bcast] + [
                        (lambda c: lambda: do_coo(c))(c) for c in range(CO)
                    ] + [do_store]

                for ib in range(NIB):
                    isl = slice(ib * IB, (ib + 1) * IB)
                    xres = xloc[:, :, isl]
                    ops = [
                        po.tile([P, IB], F32, tag="ops", name=f"ops{ib}_{i}")
                        for i in range(CO)
                    ]
                    dns = pdns.tile([16, IB], F32, tag="dns")

                    # Software-pipelined emission: PE executes in program
                    # order, so scores(jb) are emitted BEFORE the previous
                    # block's O/denominator matmuls — scores(jb) can then run
                    # while ACT is still exponentiating block jb-1, keeping
                    # ACT back-to-back (the loop's bottleneck).
                    def emit_scores(jb):
                        # two single-bank score tiles (separate pool slots so
                        # the next block's h0 scores only wait on THIS h0's
                        # exp read, not both halves)
                        e8 = esb.tile([P, 2, IB], FP8, tag="e8",
                                      name=f"e8_{ib}_{jb}")
                        for h in range(2):
                            spt = pspt.tile([P, IB], F32, tag="spt",
                                            name=f"spt{ib}_{jb}_{h}")
                            for q in range(2):
                                nc.tensor.matmul(
                                    spt,
                                    lhsT=K8[:, 2 * q : 2 * q + 2,
                                            jb * JB + h * P : jb * JB
                                            + (h + 1) * P],
                                    rhs=Q8[:, 2 * q : 2 * q + 2, isl],
                                    start=(q == 0), stop=(q == 1),
                                    perf_mode=DR,
                                )
                            nc.scalar.activation(
                                out=e8[:, h, :], in_=spt, func=Exp,
                                scale=SCALE, bias=mshift,
                            )
                        return e8

                    def emit_ov(jb, e8):
                        for cio in range(CO):
                            nc.tensor.matmul(
                                ops[cio],
                                lhsT=VT8[:, jb, :, cio * P : (cio + 1) * P],
                                rhs=e8,
                                start=(jb == 0), stop=(jb == NJB - 1),
                                perf_mode=DR,
                            )
                        nc.tensor.matmul(
                            dns, lhsT=ones8, rhs=e8,
                            start=(jb == 0), stop=(jb == NJB - 1),
                            perf_mode=DR,
                        )

                    pending = None
                    for jb in range(NJB):
                        e8 = emit_scores(jb)
                        if pending is not None:
                            emit_ov(*pending)
                        if tail:
                            tail.pop(0)()
                        pending = (jb, e8)
                    emit_ov(*pending)
                    while tail:
                        tail.pop(0)()

                    last = ib == NIB - 1
                    # drain O to SBUF fp8 (frees the 4 O banks for ib+1):
                    # one paired cast per 2-bank tile
                    O8 = att.tile([P, CO, IB], FP8, tag="O8")
                    for cio in range(CO):
                        if last and cio % 2 == 0:
                            nc.scalar.activation(
                                out=O8[:, cio, :], in_=ops[cio],
                                func=Identity, scale=OSC, bias=zero_b,
                            )
                        else:
                            nc.vector.tensor_scalar(
                                out=O8[:, cio, :], in0=ops[cio], scalar1=OSC,
                                scalar2=None, op0=mybir.AluOpType.mult,
                            )
                    recip = att.tile([1, IB], F32R, tag="recip")
                    with nc.allow_low_precision(reason="f32r holds fp32 bits"):
                        nc.vector.reciprocal(out=recip, in_=dns[0:1, :])
                    bcast_sb = att.tile([P, IB], F32, tag="bcast_sb")
                    ysb = att.tile([P, CO, IB], F32, tag="ysb")
                    if not last:
                        items = emit_out_tail(ib, O8, bcast_sb, ysb, xres,
                                              isl)
                        items[0] = (lambda f, r: lambda: f(r))(items[0],
                                                              recip)
                        tail = items
                    else:
                        # final block: nothing left to overlap with — use the
                        # freed O banks for a parallel out-projection and
                        # stream y out in half-blocks
                        bcast_ps = pfb.tile([P, IB], F32, tag="fps",
                                            name="bc_last")
                        nc.tensor.matmul(
                            bcast_ps, lhsT=ones_row, rhs=recip, start=True,
                            stop=True,
                        )
                        nc.vector.tensor_scalar(
                            out=bcast_sb, in0=bcast_ps, scalar1=128.0,
                            scalar2=None, op0=mybir.AluOpType.mult,
                        )
                        for coo in range(CO):
                            fps = po.tile([P, IB], F32, tag="ops",
                                          name=f"fpsL_{coo}")
                            for q in range(2):
                                nc.tensor.matmul(
                                    fps,
                                    lhsT=Wo8[:, 2 * q : 2 * q + 2,
                                             coo * P : (coo + 1) * P],
                                    rhs=O8[:, 2 * q : 2 * q + 2, :],
                                    start=(q == 0), stop=(q == 1),
                                    perf_mode=DR,
                                )
                            t = ysb[:, coo, :]
                            nc.vector.tensor_tensor(
                                out=t, in0=fps, in1=bcast_sb,
                                op=mybir.AluOpType.mult,
                            )
                            if coo % 2 == 0:
                                nc.vector.scalar_tensor_tensor(
                                    out=t, in0=t,
                                    scalar=bof[:, coo : coo + 1],
                                    in1=xres[:, coo, :],
                                    op0=mybir.AluOpType.add,
                                    op1=mybir.AluOpType.add,
                                )
                            else:
                                nc.gpsimd.scalar_tensor_tensor(
                                    out=t, in0=t,
                                    scalar=bof[:, coo : coo + 1],
                                    in1=xres[:, coo, :],
                                    op0=mybir.AluOpType.add,
                                    op1=mybir.AluOpType.add,
                                )
                            if coo % 2 == 1:
                                nc.sync.dma_start(
                                    out=y_t[:, coo - 1 : coo + 1, isl],
                                    in_=ysb[:, coo - 1 : coo + 1, :],
                                )
                while tail:
                    tail.pop(0)()

    nc.compile()
    return nc


def get_program(reps: int = 1):
    key = f"nc{reps}"
    if key not in _cached:
        _cached[key] = build_program(reps)
    return _cached[key]


def make_in_maps(inputs):
    x = np.asarray(inputs["x"], np.float32).reshape(B, C, HW)
    common = {
        k: np.ascontiguousarray(np.asarray(inputs[k], np.float32))
        for k in ("bq", "bk", "bv", "bo", "gamma", "beta")
    }
    for k in ("wq", "wk", "wv", "wo"):
        common[k + "t"] = np.ascontiguousarray(np.asarray(inputs[k], np.float32).T)
    in_maps = []
    for core in range(NCORES):
        b, h = core // 2, core % 2
        loc = x[b][:, h * L : (h + 1) * L]
        oth = x[b][:, (1 - h) * L : (2 - h) * L]
        xf_rot = np.ascontiguousarray(np.concatenate([loc, oth], axis=1))
        m = dict(common)
        m["xf"] = xf_rot
        in_maps.append(m)
    return in_maps


def kernel(**inputs) -> np.ndarray:
    from concourse.bass_utils import run_bass_kernel_spmd

    nc = get_program()
    in_maps = make_in_maps(inputs)
    res = run_bass_kernel_spmd(nc, in_maps, list(range(NCORES)))
    out = np.empty((B, C, HW), np.float32)
    for core in range(NCORES):
        b, h = core // 2, core % 2
        out[b][:, h * L : (h + 1) * L] = res.results[core]["y"]
    return out.reshape(B, C, 64, 64)


# revision 50
# speedup vs baseline: 1.0166x; 1.0166x over previous
"""AttentionBlock (GroupNorm -> QKV 1x1 -> single-head attention -> out proj -> residual)
for x:(4,512,64,64) f32, distributed over 8 NeuronCores.

Sharding: data-parallel over batch, 2 cores per sample, each core owns 2048 of
the 4096 query positions. Each core receives a column-ROTATED copy of its
sample (local 2048 positions first) so the compiled program is identical on
every core (SPMD).

Numerics/engine strategy (fp8e4m3 + DoubleRow for all heavy matmuls; fits the
rel-err budget with ~4x margin per a host-side quantization study):
  1) stats stream: x chunks HBM->SBUF f32; DVE bn_stats; ACT casts x into an
     fp8 SBUF-resident copy. Cross-partition group reduction via PE transposes
     yields the per-channel GroupNorm affine (a, b): xn = a*x + b.
  2) weights: W' = W diag(a) applied during the f32 -> fp8 cast (per-partition
     scale). Biases fold entirely into per-channel vectors: b'_q = bq + Wq b
     (tiny N=1 f32 matmuls, channel layout); the K bias is DROPPED (it adds a
     per-query constant to logits, exactly cancelled by softmax); the V bias
     folds THROUGH the output projection: bo' = bo + Wo (bv + Wv b), so the
     V/O path never adds a bias in the hot loop.
  3) Q/K/V projections: fp8 DoubleRow matmuls (256-deep contraction per
     instruction, 0.5 cyc/row); PSUM -> SBUF casts to fp8 rotate across
     DVE/ACT (GPSIMD cannot read PSUM; it handles SBUF-side adds + DMA
     issue only). Score matmuls are software-pipelined ahead of the previous
     block's O matmuls so the in-order PE queue never blocks ACT.
  4) Attention per 512-query block over 256-key blocks: scores K^T Q fp8-DR
     into a 2-bank PSUM tile; ACT exp (scale, constant max-shift -4, softmax-
     invariant, keeps E in fp8 range) emits E fp8; O += V^T E fp8-DR; the
     softmax denominator accumulates in PSUM via an fp8 ones-DR matmul
     (no vector-engine reduction). PSUM: 4 (O) + 2 (scores) + 1 (denom) +
     1 (bcast/out-proj) = 8 banks.
  5) late normalization: O is drained to fp8 unnormalized (pre-scaled 1/128),
     freeing its PSUM banks immediately; out = (Wo O) * (128/denom) + bo' + x
     with the residual taken from the SBUF-resident f32 local half of x. The
     output tail's PE work is interleaved into the next query block's loop.
"""

import sys

sys.path.insert(0, "/opt/trn_rl_repo")

import numpy as np
from contextlib import ExitStack

import concourse.bass as bass
import concourse.tile as tile
from concourse import bacc, mybir
from concourse.masks import make_identity

F32 = mybir.dt.float32
F32R = mybir.dt.float32r
FP8 = mybir.dt.float8e4
DR = mybir.MatmulPerfMode.DoubleRow
Exp = mybir.ActivationFunctionType.Exp
Identity = mybir.ActivationFunctionType.Identity

C = 512          # channels
HW = 4096        # spatial positions per sample
L = 2048         # query positions per core
P = 128          # partitions
CO = C // P      # 4 channel chunks
NG = 32          # groups
GS = C // NG     # 16 channels per group
G_PER_CO = P // GS  # 8 groups per 128-partition chunk
EPS = 1e-6
SCALE = C ** -0.5
MSHIFT = 4.0     # constant subtracted inside exp (softmax-invariant)
IB = 512         # query block
NIB = L // IB    # 4
JB = 256         # key block per attention step
NJB = HW // JB   # 16
NXC = HW // 512  # 8 x-stream chunks
B = 4            # batch
NCORES = 8

_cached = {}


def build_program(reps: int = 1):
    nc = bacc.Bacc(None, target_bir_lowering=False)

    xf = nc.declare_dram_parameter("xf", [C, HW], F32, isOutput=False)
    wqt_d = nc.declare_dram_parameter("wqt", [C, C], F32, isOutput=False)
    wkt_d = nc.declare_dram_parameter("wkt", [C, C], F32, isOutput=False)
    wvt_d = nc.declare_dram_parameter("wvt", [C, C], F32, isOutput=False)
    wot_d = nc.declare_dram_parameter("wot", [C, C], F32, isOutput=False)
    bq_d = nc.declare_dram_parameter("bq", [C], F32, isOutput=False)
    bk_d = nc.declare_dram_parameter("bk", [C], F32, isOutput=False)
    bv_d = nc.declare_dram_parameter("bv", [C], F32, isOutput=False)
    bo_d = nc.declare_dram_parameter("bo", [C], F32, isOutput=False)
    gamma_d = nc.declare_dram_parameter("gamma", [C], F32, isOutput=False)
    beta_d = nc.declare_dram_parameter("beta", [C], F32, isOutput=False)
    y = nc.declare_dram_parameter("y", [C, L], F32, isOutput=True)

    # [c, j] -> [cp, coo, j] with c = coo*128 + cp
    xf_t = xf[:].rearrange("(coo cp) j -> cp coo j", cp=P)
    y_t = y[:].rearrange("(coo cp) i -> cp coo i", cp=P)

    with tile.TileContext(nc) as tc:
        for _rep in range(reps):
          with ExitStack() as ctx:
            consts = ctx.enter_context(tc.tile_pool(name="consts", bufs=1))
            big = ctx.enter_context(tc.tile_pool(name="big", bufs=1))
            po = ctx.enter_context(tc.psum_pool(name="po", bufs=4))

            ident = consts.tile([P, P], F32)
            make_identity(nc, ident)
            onesf = consts.tile([P, 2, 16], F32)
            nc.vector.memset(onesf, 1.0)
            ones8 = consts.tile([P, 2, 16], FP8)
            nc.vector.tensor_copy(out=ones8, in_=onesf)
            ones_rowf = consts.tile([1, P], F32)
            nc.vector.memset(ones_rowf, 1.0)
            ones_row = consts.tile([1, P], F32R)
            nc.vector.tensor_copy(out=ones_row, in_=ones_rowf)
            mshift = consts.tile([P, 1], F32)
            nc.vector.memset(mshift, -MSHIFT)
            zero_b = consts.tile([P, 1], F32)
            nc.vector.memset(zero_b, 0.0)
            epsg_t = consts.tile([P, 1], F32)
            nc.vector.memset(epsg_t, EPS * GS * GS)
            lngs_t = consts.tile([P, 1], F32)
            nc.vector.memset(lngs_t, float(np.log(GS)))

            def load_chan_vec(name, dsrc):
                t = consts.tile([P, CO], F32, tag=name)
                # gpsimd (SWDGE) issue: keeps SP free for the x stream
                nc.gpsimd.dma_start(
                    out=t, in_=dsrc[:].rearrange("(coo cp) -> cp coo", cp=P)
                )
                return t

            gamma_sb = load_chan_vec("gamma_sb", gamma_d)
            beta_sb = load_chan_vec("beta_sb", beta_d)
            bo_ch = load_chan_vec("bo_ch", bo_d)
            bq_ch = load_chan_vec("bq_ch", bq_d)
            bv_ch = load_chan_vec("bv_ch", bv_d)

            # fp8 operand stores (SBUF-resident); xloc keeps the local half
            # of x in f32 for the residual add (saves a 4MB re-stream)
            x8 = big.tile([P, CO, HW], FP8, tag="x8")
            xloc = big.tile([P, CO, L], F32, tag="xloc")
            K8 = big.tile([P, CO, HW], FP8, tag="K8")
            VT8 = big.tile([P, NJB, 2, C], FP8, tag="VT8")
            Q8 = big.tile([P, CO, L], FP8, tag="Q8")
            Wq8 = big.tile([P, CO, C], FP8, tag="Wq8")
            Wk8 = big.tile([P, CO, C], FP8, tag="Wk8")
            Wv8 = big.tile([P, CO, C], FP8, tag="Wv8")
            Wo8 = big.tile([P, CO, C], FP8, tag="Wo8")
            bqf = consts.tile([P, CO], F32, tag="bqf")
            bvf = consts.tile([P, CO], F32, tag="bvf")
            bof = consts.tile([P, CO], F32, tag="bof")

            with ExitStack() as phctx:
                phW = phctx.enter_context(tc.tile_pool(name="phW", bufs=1))
                ph1 = phctx.enter_context(tc.tile_pool(name="ph1", bufs=1))
                xchunk = phctx.enter_context(tc.tile_pool(name="xchunk",
                                                          bufs=2))
                pp1 = phctx.enter_context(tc.psum_pool(name="pp1", bufs=2))

                WqT = phW.tile([P, CO, C], F32, tag="WqT")
                WkT = phW.tile([P, CO, C], F32, tag="WkT")
                WvT = phW.tile([P, CO, C], F32, tag="WvT")
                WoT = phW.tile([P, CO, C], F32, tag="WoT")
                for WT, wsrc in (
                    (WqT, wqt_d), (WkT, wkt_d), (WvT, wvt_d), (WoT, wot_d)
                ):
                    # ACT-queue issue: SP stays dedicated to the x stream
                    nc.scalar.dma_start(
                        out=WT,
                        in_=wsrc[:].rearrange("(cio cp) co -> cp cio co",
                                              cp=P),
                    )
                # ---------- Phase A: stats stream + x -> fp8 ----------
                # x arrives in 4 DMAs of 1024 positions; the first two land in
                # the resident xloc (local query half, reused as the residual)
                stats = ph1.tile([P, CO, NXC, 6], F32, tag="stats")
                for ci in range(4):
                    csl = slice(ci * 1024, (ci + 1) * 1024)
                    if ci < 2:
                        xc = xloc[:, :, ci * 1024 : (ci + 1) * 1024]
                    else:
                        xc = xchunk.tile([P, CO, 1024], F32, tag="xc",
                                         name=f"xcs{ci}")
                    nc.sync.dma_start(out=xc, in_=xf_t[:, :, csl])
                    for coo in range(CO):
                        for hf in range(2):
                            nc.vector.bn_stats(
                                out=stats[:, coo, 2 * ci + hf, :],
                                in_=xc[:, coo, hf * 512 : (hf + 1) * 512],
                            )
                    nc.gpsimd.tensor_copy(out=x8[:, :, csl], in_=xc)
                mv = ph1.tile([P, CO, 2], F32, tag="mv")
                for coo in range(CO):
                    nc.vector.bn_aggr(out=mv[:, coo, :],
                                      in_=stats[:, coo, :, :])
                # T_in cols 0:4 per-channel mean, 4:8 per-channel E[x^2]
                T_in = ph1.tile([P, 8], F32, tag="T_in")
                nc.vector.tensor_copy(T_in[:, 0:CO], mv[:, :, 0])
                nc.vector.tensor_tensor(
                    out=T_in[:, CO : 2 * CO], in0=mv[:, :, 0], in1=mv[:, :, 0],
                    op=mybir.AluOpType.mult,
                )
                nc.vector.tensor_tensor(
                    out=T_in[:, CO : 2 * CO], in0=T_in[:, CO : 2 * CO],
                    in1=mv[:, :, 1], op=mybir.AluOpType.add,
                )
                tps = pp1.tile([8, P], F32, tag="wtp")
                nc.tensor.transpose(tps, T_in, ident)
                T_sb = ph1.tile([8, P], F32, tag="T_sb")
                nc.vector.tensor_copy(T_sb, tps)
                # G rows 0:4 = group-sums of channel means, 4:8 of E[x^2]
                G = ph1.tile([8, G_PER_CO], F32, tag="G")
                nc.vector.reduce_sum(
                    out=G, in_=T_sb.rearrange("p (g s) -> p g s", s=GS),
                    axis=mybir.AxisListType.X,
                )
                # broadcast group values along channels, then ONE transpose
                # back to channel layout — no cross-partition DMA bounce
                B2 = ph1.tile([8, P], F32, tag="B2")
                bc = bass.AP(
                    tensor=G.tensor, offset=G.offset,
                    ap=[G.ap[0], G.ap[1], [0, GS]],
                )
                nc.vector.tensor_copy(
                    B2.rearrange("p (g s) -> p g s", s=GS), bc
                )
                tc_ps = pp1.tile([P, 8], F32, tag="wtp", name="tc_ps")
                nc.tensor.transpose(tc_ps, B2, ident[0:8, 0:8])
                Tc = ph1.tile([P, 8], F32, tag="Tc")
                nc.vector.tensor_copy(Tc, tc_ps)
                Msum = Tc[:, 0:CO]       # per-group sum of means
                Ssum = Tc[:, CO : 2 * CO]  # per-group sum of E[x^2]
                # var*GS^2 = Ssum*GS - Msum^2 ;  rstd = GS/sqrt(var*GS^2 +
                # eps*GS^2) = exp(-0.5*ln(.) + ln(GS))
                M2 = ph1.tile([P, CO], F32, tag="M2")
                nc.vector.tensor_tensor(
                    out=M2, in0=Msum, in1=Msum, op=mybir.AluOpType.mult
                )
                Vg = ph1.tile([P, CO], F32, tag="Vg")
                nc.vector.scalar_tensor_tensor(
                    out=Vg, in0=Ssum, scalar=float(GS), in1=M2,
                    op0=mybir.AluOpType.mult, op1=mybir.AluOpType.subtract,
                )
                rstd_ch = ph1.tile([P, CO], F32, tag="rstd_ch")
                nc.scalar.activation(
                    out=rstd_ch, in_=Vg,
                    func=mybir.ActivationFunctionType.Ln,
                    bias=epsg_t, scale=1.0,
                )
                nc.scalar.activation(
                    out=rstd_ch, in_=rstd_ch, func=Exp, scale=-0.5,
                    bias=lngs_t,
                )
                a_ch = consts.tile([P, CO], F32, tag="a_ch")
                b_ch = consts.tile([P, CO], F32, tag="b_ch")
                nc.vector.tensor_tensor(
                    out=a_ch, in0=gamma_sb, in1=rstd_ch,
                    op=mybir.AluOpType.mult,
                )
                # b = beta - (Msum/GS) * a
                nc.vector.tensor_tensor(
                    out=b_ch, in0=Msum, in1=a_ch, op=mybir.AluOpType.mult
                )
                nc.vector.scalar_tensor_tensor(
                    out=b_ch, in0=b_ch, scalar=-1.0 / GS, in1=beta_sb,
                    op0=mybir.AluOpType.mult, op1=mybir.AluOpType.add,
                )

                # ---------- Phase B: bias folds + weight scale/cast ----------
                # b'_w = b_w + W b_ch in channel layout via N=1 f32 matmuls
                def fold_bias(WT, rhs_col_src, base_vec, bdst, nm):
                    for coo in range(CO):
                        pb = pp1.tile([P, 1], F32, tag="pb",
                                      name=f"pb_{nm}{coo}")
                        for cio in range(CO):
                            nc.tensor.matmul(
                                pb,
                                lhsT=WT[:, cio, coo * P : (coo + 1) * P],
                                rhs=rhs_col_src[:, cio : cio + 1],
                                start=(cio == 0), stop=(cio == CO - 1),
                            )
                        nc.vector.tensor_tensor(
                            out=bdst[:, coo : coo + 1], in0=pb,
                            in1=base_vec[:, coo : coo + 1],
                            op=mybir.AluOpType.add,
                        )

                fold_bias(WqT, b_ch, bq_ch, bqf, "q")
                # K bias dropped: it only adds a per-query constant to the
                # logits, which softmax over keys cancels exactly
                fold_bias(WvT, b_ch, bv_ch, bvf, "v")
                # V bias folds through the output projection:
                # bo' = bo + Wo b'v
                fold_bias(WoT, bvf, bo_ch, bof, "o")

                # W'8 = fp8(W * a) — per-input-channel scale; split engines
                for cio in range(CO):
                    nc.vector.tensor_scalar_mul(
                        Wq8[:, cio, :], WqT[:, cio, :], a_ch[:, cio : cio + 1]
                    )
                    nc.scalar.activation(
                        out=Wk8[:, cio, :], in_=WkT[:, cio, :], func=Identity,
                        scale=a_ch[:, cio : cio + 1], bias=zero_b,
                    )
                    nc.gpsimd.tensor_scalar_mul(
                        Wv8[:, cio, :], WvT[:, cio, :], a_ch[:, cio : cio + 1]
                    )
                nc.vector.tensor_copy(out=Wo8, in_=WoT)

                # PSUM->SBUF fp8 cast with per-partition bias, engine-rotated
                # GPSIMD cannot read PSUM (walrus birverifier) — PSUM->SBUF
                # casts run on DVE/ACT only
                def cast_bias(dst, src, bias_ap, eng):
                    if eng == 0:
                        nc.vector.tensor_scalar(
                            out=dst, in0=src, scalar1=bias_ap, scalar2=None,
                            op0=mybir.AluOpType.add,
                        )
                    else:
                        nc.scalar.activation(
                            out=dst, in_=src, func=Identity, bias=bias_ap,
                            scale=1.0,
                        )

                # ---------- Phase C: Q projection (fp8 DR) ----------
                nce = 0
                for coo in range(CO):
                    for ib in range(NIB):
                        isl = slice(ib * IB, (ib + 1) * IB)
                        pq = po.tile([P, IB], F32, tag="ops",
                                     name=f"pq{ib}_{coo}")
                        for q in range(2):
                            nc.tensor.matmul(
                                pq,
                                lhsT=Wq8[:, 2 * q : 2 * q + 2,
                                         coo * P : (coo + 1) * P],
                                rhs=x8[:, 2 * q : 2 * q + 2, isl],
                                start=(q == 0), stop=(q == 1), perf_mode=DR,
                            )
                        cast_bias(Q8[:, coo, isl], pq,
                                  bqf[:, coo : coo + 1], nce % 2)
                        nce += 1

                # ---------- Phase D: K and V^T, interleaved per 512-col
                # block so attention consumption order matches production ----
                for s in range(NXC):
                    jsl = slice(s * 512, (s + 1) * 512)
                    for coo in range(CO):
                        pk = po.tile([P, 512], F32, tag="ops",
                                     name=f"pk{s}_{coo}")
                        for q in range(2):
                            nc.tensor.matmul(
                                pk,
                                lhsT=Wk8[:, 2 * q : 2 * q + 2,
                                         coo * P : (coo + 1) * P],
                                rhs=x8[:, 2 * q : 2 * q + 2, jsl],
                                start=(q == 0), stop=(q == 1), perf_mode=DR,
                            )
                        # no bias (dropped: softmax-invariant): pure cast
                        dstk = K8[:, coo, jsl]
                        eng = nce % 2
                        nce += 1
                        if eng == 0:
                            nc.vector.tensor_copy(out=dstk, in_=pk)
                        else:
                            nc.scalar.activation(
                                out=dstk, in_=pk, func=Identity, scale=1.0,
                                bias=zero_b,
                            )
                    for jb in (2 * s, 2 * s + 1):
                        for jh in range(2):
                            jpos = jb * JB + jh * P
                            pv = po.tile([P, C], F32, tag="ops",
                                         name=f"pv{jb}_{jh}")
                            for q in range(2):
                                nc.tensor.matmul(
                                    pv,
                                    lhsT=x8[:, 2 * q : 2 * q + 2,
                                            jpos : jpos + P],
                                    rhs=Wv8[:, 2 * q : 2 * q + 2, :],
                                    start=(q == 0), stop=(q == 1),
                                    perf_mode=DR,
                                )
                            dstv = VT8[:, jb, jh, :]
                            eng = nce % 2
                            nce += 1
                            if eng == 0:
                                nc.vector.tensor_copy(out=dstv, in_=pv)
                            else:
                                nc.scalar.activation(
                                    out=dstv, in_=pv, func=Identity,
                                    scale=1.0, bias=zero_b,
                                )

            # ---------- Phase E: attention + output ----------
            # Late normalization: O is cast to fp8 unnormalized (pre-scaled
            # by 1/128 to stay in fp8 range) the moment the j-loop ends,
            # releasing the 4 O PSUM banks immediately; the output projection
            # runs on unnormalized O and the 128/denom factor is applied in
            # the final DVE op. The out-projection PE work is interleaved
            # into the NEXT query block's pipeline slots so the in-order PE
            # queue never stalls on the output tail.
            OSC = 1.0 / 128.0
            with (
                tc.tile_pool(name="att", bufs=2) as att,
                tc.tile_pool(name="esb", bufs=4) as esb,
                tc.psum_pool(name="pspt", bufs=2) as pspt,
                tc.psum_pool(name="pdns", bufs=1) as pdns,
                tc.psum_pool(name="pfb", bufs=1) as pfb,
            ):
                tail = []  # pending PE op-groups from the previous ib

                def gp_bias_res_add(t, coo, xres):
                    # t += bo' (free-broadcast AP) ; t += x  — SBUF-only ops
                    # on the otherwise idle GPSIMD engine
                    bof_b = bass.AP(
                        tensor=bof.tensor, offset=bof.offset + coo,
                        ap=[bof.ap[0], [0, IB]],
                    )
                    nc.gpsimd.tensor_tensor(
                        out=t, in0=t, in1=bof_b, op=mybir.AluOpType.add,
                    )
                    nc.gpsimd.tensor_tensor(
                        out=t, in0=t, in1=xres[:, coo, :],
                        op=mybir.AluOpType.add,
                    )


                def emit_out_tail(ib, O8, bcast_sb, ysb, xres, isl):
                    def do_bcast(recip=None, _ib=ib):
                        bcast_ps = pfb.tile([P, IB], F32, tag="fps",
                                            name=f"bc{_ib}")
                        nc.tensor.matmul(
                            bcast_ps, lhsT=ones_row, rhs=recip, start=True,
                            stop=True,
                        )
                        # 128/denom: undo the 1/128 O pre-scale
                        nc.vector.tensor_scalar(
                            out=bcast_sb, in0=bcast_ps, scalar1=128.0,
                            scalar2=None, op0=mybir.AluOpType.mult,
                        )

                    def do_coo(coo, _ib=ib):
                        fps = pfb.tile([P, IB], F32, tag="fps",
                                       name=f"fps{_ib}_{coo}")
                        for q in range(2):
                            nc.tensor.matmul(
                                fps,
                                lhsT=Wo8[:, 2 * q : 2 * q + 2,
                                         coo * P : (coo + 1) * P],
                                rhs=O8[:, 2 * q : 2 * q + 2, :],
                                start=(q == 0), stop=(q == 1), perf_mode=DR,
                            )
                        t = ysb[:, coo, :]
                        nc.vector.tensor_tensor(
                            out=t, in0=fps, in1=bcast_sb,
                            op=mybir.AluOpType.mult,
                        )
                        if coo % 2 == 0:
                            nc.vector.scalar_tensor_tensor(
                                out=t, in0=t, scalar=bof[:, coo : coo + 1],
                                in1=xres[:, coo, :],
                                op0=mybir.AluOpType.add,
                                op1=mybir.AluOpType.add,
                            )
                        else:
                            nc.gpsimd.scalar_tensor_tensor(
                                out=t, in0=t, scalar=bof[:, coo : coo + 1],
                                in1=xres[:, coo, :],
                                op0=mybir.AluOpType.add,
                                op1=mybir.AluOpType.add,
                            )

                    def do_store(_isl=isl, _ysb=ysb):
                        nc.sync.dma_start(out=y_t[:, :, _isl], in_=_ysb)

                    return [do_bcast] + [
                        (lambda c: lambda: do_coo(c))(c) for c in range(CO)
                    ] + [do_store]

                for ib in range(NIB):
                    isl = slice(ib * IB, (ib + 1) * IB)
                    xres = xloc[:, :, isl]
                    ops = [
                        po.tile([P, IB], F32, tag="ops", name=f"ops{ib}_{i}")
                        for i in range(CO)
                    ]
                    dns = pdns.tile([16, IB], F32, tag="dns")

                    # Software-pipelined emission: PE executes in program
                    # order, so scores(jb) are emitted BEFORE the previous
                    # block's O/denominator matmuls — scores(jb) can then run
                    # while ACT is still exponentiating block jb-1, keeping
                    # ACT back-to-back (the loop's bottleneck).
                    def emit_scores(jb):
                        # two single-bank score tiles (separate pool slots so
                        # the next block's h0 scores only wait on THIS h0's
                        # exp read, not both halves)
                        e8 = esb.tile([P, 2, IB], FP8, tag="e8",
                                      name=f"e8_{ib}_{jb}")
                        for h in range(2):
                            spt = pspt.tile([P, IB], F32, tag="spt",
                                            name=f"spt{ib}_{jb}_{h}")
                            for q in range(2):
                                nc.tensor.matmul(
                                    spt,
                                    lhsT=K8[:, 2 * q : 2 * q + 2,
                                            jb * JB + h * P : jb * JB
                                            + (h + 1) * P],
                                    rhs=Q8[:, 2 * q : 2 * q + 2, isl],
                                    start=(q == 0), stop=(q == 1),
                                    perf_mode=DR,
                                )
                            nc.scalar.activation(
                                out=e8[:, h, :], in_=spt, func=Exp,
                                scale=SCALE, bias=mshift,
                            )
                        return e8

                    def emit_ov(jb, e8):
                        for cio in range(CO):
                            nc.tensor.matmul(
                                ops[cio],
                                lhsT=VT8[:, jb, :, cio * P : (cio + 1) * P],
                                rhs=e8,
                                start=(jb == 0), stop=(jb == NJB - 1),
                                perf_mode=DR,
                            )
                        nc.tensor.matmul(
                            dns, lhsT=ones8, rhs=e8,
                            start=(jb == 0), stop=(jb == NJB - 1),
                            perf_mode=DR,
                        )

                    pending = None
                    for jb in range(NJB):
                        e8 = emit_scores(jb)
                        if pending is not None:
                            emit_ov(*pending)
                        if tail:
                            tail.pop(0)()
                        pending = (jb, e8)
                    emit_ov(*pending)
                    while tail:
                        tail.pop(0)()

                    last = ib == NIB - 1
                    # drain O to SBUF fp8 (frees the 4 O banks for ib+1):
                    # one paired cast per 2-bank tile
                    O8 = att.tile([P, CO, IB], FP8, tag="O8")
                    for cio in range(CO):
                        if last and cio % 2 == 0:
                            nc.scalar.activation(
                                out=O8[:, cio, :], in_=ops[cio],
                                func=Identity, scale=OSC, bias=zero_b,
                            )
                        else:
                            nc.vector.tensor_scalar(
                                out=O8[:, cio, :], in0=ops[cio], scalar1=OSC,
                                scalar2=None, op0=mybir.AluOpType.mult,
                            )
                    recip = att.tile([1, IB], F32R, tag="recip")
                    with nc.allow_low_precision(reason="f32r holds fp32 bits"):
                        nc.vector.reciprocal(out=recip, in_=dns[0:1, :])
                    bcast_sb = att.tile([P, IB], F32, tag="bcast_sb")
                    ysb = att.tile([P, CO, IB], F32, tag="ysb")
                    if not last:
                        items = emit_out_tail(ib, O8, bcast_sb, ysb, xres,
                                              isl)
                        items[0] = (lambda f, r: lambda: f(r))(items[0],
                                                              recip)
                        tail = items
                    else:
                        # final block: nothing left to overlap with — use the
                        # freed O banks for a parallel out-projection and
                        # stream y out in half-blocks
                        bcast_ps = pfb.tile([P, IB], F32, tag="fps",
                                            name="bc_last")
                        nc.tensor.matmul(
                            bcast_ps, lhsT=ones_row, rhs=recip, start=True,
                            stop=True,
                        )
                        nc.vector.tensor_scalar(
                            out=bcast_sb, in0=bcast_ps, scalar1=128.0,
                            scalar2=None, op0=mybir.AluOpType.mult,
                        )
                        for coo in range(CO):
                            fps = po.tile([P, IB], F32, tag="ops",
                                          name=f"fpsL_{coo}")
                            for q in range(2):
                                nc.tensor.matmul(
                                    fps,
                                    lhsT=Wo8[:, 2 * q : 2 * q + 2,
                                             coo * P : (coo + 1) * P],
                                    rhs=O8[:, 2 * q : 2 * q + 2, :],
                                    start=(q == 0), stop=(q == 1),
                                    perf_mode=DR,
                                )
                            t = ysb[:, coo, :]
                            nc.vector.tensor_tensor(
                                out=t, in0=fps, in1=bcast_sb,
                                op=mybir.AluOpType.mult,
                            )
                            if coo % 2 == 0:
                                nc.vector.scalar_tensor_tensor(
                                    out=t, in0=t,
                                    scalar=bof[:, coo : coo + 1],
                                    in1=xres[:, coo, :],
                                    op0=mybir.AluOpType.add,
                                    op1=mybir.AluOpType.add,
                                )
                            else:
                                nc.gpsimd.scalar_tensor_tensor(
                                    out=t, in0=t,
                                    scalar=bof[:, coo : coo + 1],
                                    in1=xres[:, coo, :],
                                    op0=mybir.AluOpType.add,
                                    op1=mybir.AluOpType.add,
                                )
                            if coo % 2 == 1:
                                nc.sync.dma_start(
                                    out=y_t[:, coo - 1 : coo + 1, isl],
                                    in_=ysb[:, coo - 1 : coo + 1, :],
                                )
                while tail:
                    tail.pop(0)()

    nc.compile()
    return nc


def get_program(reps: int = 1):
    key = f"nc{reps}"
    if key not in _cached:
        _cached[key] = build_program(reps)
    return _cached[key]


def make_in_maps(inputs):
    x = np.asarray(inputs["x"], np.float32).reshape(B, C, HW)
    common = {
        k: np.ascontiguousarray(np.asarray(inputs[k], np.float32))
        for k in ("bq", "bk", "bv", "bo", "gamma", "beta")
    }
    for k in ("wq", "wk", "wv", "wo"):
        common[k + "t"] = np.ascontiguousarray(np.asarray(inputs[k], np.float32).T)
    in_maps = []
    for core in range(NCORES):
        b, h = core // 2, core % 2
        loc = x[b][:, h * L : (h + 1) * L]
        oth = x[b][:, (1 - h) * L : (2 - h) * L]
        xf_rot = np.ascontiguousarray(np.concatenate([loc, oth], axis=1))
        m = dict(common)
        m["xf"] = xf_rot
        in_maps.append(m)
    return in_maps


def kernel(**inputs) -> np.ndarray:
    from concourse.bass_utils import run_bass_kernel_spmd

    nc = get_program()
    in_maps = make_in_maps(inputs)
    res = run_bass_kernel_spmd(nc, in_maps, list(range(NCORES)))
    out = np.empty((B, C, HW), np.float32)
    for core in range(NCORES):
        b, h = core // 2, core % 2
        out[b][:, h * L : (h + 1) * L] = res.results[core]["y"]
    return out.reshape(B, C, 64, 64)


# revision 54
# speedup vs baseline: 1.0239x; 1.0072x over previous
"""AttentionBlock (GroupNorm -> QKV 1x1 -> single-head attention -> out proj -> residual)
for x:(4,512,64,64) f32, distributed over 8 NeuronCores.

Sharding: data-parallel over batch, 2 cores per sample, each core owns 2048 of
the 4096 query positions. Each core receives a column-ROTATED copy of its
sample (local 2048 positions first) so the compiled program is identical on
every core (SPMD).

Numerics/engine strategy (fp8e4m3 + DoubleRow for all heavy matmuls; fits the
rel-err budget with ~4x margin per a host-side quantization study):
  1) stats stream: x chunks HBM->SBUF f32; DVE bn_stats; ACT casts x into an
     fp8 SBUF-resident copy. Cross-partition group reduction via PE transposes
     yields the per-channel GroupNorm affine (a, b): xn = a*x + b.
  2) weights: W' = W diag(a) applied during the f32 -> fp8 cast (per-partition
     scale). Biases fold entirely into per-channel vectors: b'_q = bq + Wq b
     (tiny N=1 f32 matmuls, channel layout); the K bias is DROPPED (it adds a
     per-query constant to logits, exactly cancelled by softmax); the V bias
     folds THROUGH the output projection: bo' = bo + Wo (bv + Wv b), so the
     V/O path never adds a bias in the hot loop.
  3) Q/K/V projections: fp8 DoubleRow matmuls (256-deep contraction per
     instruction, 0.5 cyc/row); PSUM -> SBUF casts to fp8 rotate across
     DVE/ACT (GPSIMD cannot read PSUM; it handles SBUF-side adds + DMA
     issue only). Score matmuls are software-pipelined ahead of the previous
     block's O matmuls so the in-order PE queue never blocks ACT.
  4) Attention per 512-query block over 256-key blocks: scores K^T Q fp8-DR
     into a 2-bank PSUM tile; ACT exp (scale, constant max-shift -4, softmax-
     invariant, keeps E in fp8 range) emits E fp8; O += V^T E fp8-DR; the
     softmax denominator accumulates in PSUM via an fp8 ones-DR matmul
     (no vector-engine reduction). PSUM: 4 (O) + 2 (scores) + 1 (denom) +
     1 (bcast/out-proj) = 8 banks.
  5) late normalization: O is drained to fp8 unnormalized (pre-scaled 1/128),
     freeing its PSUM banks immediately; out = (Wo O) * (128/denom) + bo' + x
     with the residual taken from the SBUF-resident f32 local half of x. The
     output tail's PE work is interleaved into the next query block's loop.
"""

import sys

sys.path.insert(0, "/opt/trn_rl_repo")

import numpy as np
from contextlib import ExitStack

import concourse.bass as bass
import concourse.tile as tile
from concourse import bacc, mybir
from concourse.masks import make_identity

F32 = mybir.dt.float32
F32R = mybir.dt.float32r
FP8 = mybir.dt.float8e4
DR = mybir.MatmulPerfMode.DoubleRow
Exp = mybir.ActivationFunctionType.Exp
Identity = mybir.ActivationFunctionType.Identity

C = 512          # channels
HW = 4096        # spatial positions per sample
L = 2048         # query positions per core
P = 128          # partitions
CO = C // P      # 4 channel chunks
NG = 32          # groups
GS = C // NG     # 16 channels per group
G_PER_CO = P // GS  # 8 groups per 128-partition chunk
EPS = 1e-6
SCALE = C ** -0.5
MSHIFT = 4.0     # constant subtracted inside exp (softmax-invariant)
IB = 512         # query block
NIB = L // IB    # 4
JB = 256         # key block per attention step
NJB = HW // JB   # 16
NXC = HW // 512  # 8 x-stream chunks
B = 4            # batch
NCORES = 8

_cached = {}


def build_program(reps: int = 1):
    nc = bacc.Bacc(None, target_bir_lowering=False)

    xf = nc.declare_dram_parameter("xf", [C, HW], F32, isOutput=False)
    wqt_d = nc.declare_dram_parameter("wqt", [C, C], F32, isOutput=False)
    wkt_d = nc.declare_dram_parameter("wkt", [C, C], F32, isOutput=False)
    wvt_d = nc.declare_dram_parameter("wvt", [C, C], F32, isOutput=False)
    wot_d = nc.declare_dram_parameter("wot", [C, C], F32, isOutput=False)
    bq_d = nc.declare_dram_parameter("bq", [C], F32, isOutput=False)
    bk_d = nc.declare_dram_parameter("bk", [C], F32, isOutput=False)
    bv_d = nc.declare_dram_parameter("bv", [C], F32, isOutput=False)
    bo_d = nc.declare_dram_parameter("bo", [C], F32, isOutput=False)
    gamma_d = nc.declare_dram_parameter("gamma", [C], F32, isOutput=False)
    beta_d = nc.declare_dram_parameter("beta", [C], F32, isOutput=False)
    y = nc.declare_dram_parameter("y", [C, L], F32, isOutput=True)

    # [c, j] -> [cp, coo, j] with c = coo*128 + cp
    xf_t = xf[:].rearrange("(coo cp) j -> cp coo j", cp=P)
    y_t = y[:].rearrange("(coo cp) i -> cp coo i", cp=P)

    with tile.TileContext(nc) as tc:
        for _rep in range(reps):
          with ExitStack() as ctx:
            consts = ctx.enter_context(tc.tile_pool(name="consts", bufs=1))
            big = ctx.enter_context(tc.tile_pool(name="big", bufs=1))
            po = ctx.enter_context(tc.psum_pool(name="po", bufs=4))

            ident = consts.tile([P, P], F32)
            make_identity(nc, ident)
            onesf = consts.tile([P, 2, 16], F32)
            nc.vector.memset(onesf, 1.0)
            ones8 = consts.tile([P, 2, 16], FP8)
            nc.vector.tensor_copy(out=ones8, in_=onesf)
            ones_rowf = consts.tile([1, P], F32)
            nc.vector.memset(ones_rowf, 1.0)
            ones_row = consts.tile([1, P], F32R)
            nc.vector.tensor_copy(out=ones_row, in_=ones_rowf)
            mshift = consts.tile([P, 1], F32)
            nc.vector.memset(mshift, -MSHIFT)
            zero_b = consts.tile([P, 1], F32)
            nc.vector.memset(zero_b, 0.0)
            epsg_t = consts.tile([P, 1], F32)
            nc.vector.memset(epsg_t, EPS * GS * GS)
            lngs_t = consts.tile([P, 1], F32)
            nc.vector.memset(lngs_t, float(np.log(GS)))

            def load_chan_vec(name, dsrc):
                t = consts.tile([P, CO], F32, tag=name)
                # gpsimd (SWDGE) issue: keeps SP free for the x stream
                nc.gpsimd.dma_start(
                    out=t, in_=dsrc[:].rearrange("(coo cp) -> cp coo", cp=P)
                )
                return t

            gamma_sb = load_chan_vec("gamma_sb", gamma_d)
            beta_sb = load_chan_vec("beta_sb", beta_d)
            bo_ch = load_chan_vec("bo_ch", bo_d)
            bq_ch = load_chan_vec("bq_ch", bq_d)
            bv_ch = load_chan_vec("bv_ch", bv_d)

            # fp8 operand stores (SBUF-resident); xloc keeps the local half
            # of x in f32 for the residual add (saves a 4MB re-stream)
            x8 = big.tile([P, CO, HW], FP8, tag="x8")
            xloc = big.tile([P, CO, L], F32, tag="xloc")
            K8 = big.tile([P, CO, HW], FP8, tag="K8")
            VT8 = big.tile([P, NJB, 2, C], FP8, tag="VT8")
            Q8 = big.tile([P, CO, L], FP8, tag="Q8")
            Wq8 = big.tile([P, CO, C], FP8, tag="Wq8")
            Wk8 = big.tile([P, CO, C], FP8, tag="Wk8")
            Wv8 = big.tile([P, CO, C], FP8, tag="Wv8")
            Wo8 = big.tile([P, CO, C], FP8, tag="Wo8")
            bqf = consts.tile([P, CO], F32, tag="bqf")
            bvf = consts.tile([P, CO], F32, tag="bvf")
            bof = consts.tile([P, CO], F32, tag="bof")

            with ExitStack() as phctx:
                phW = phctx.enter_context(tc.tile_pool(name="phW", bufs=1))
                ph1 = phctx.enter_context(tc.tile_pool(name="ph1", bufs=1))
                xchunk = phctx.enter_context(tc.tile_pool(name="xchunk",
                                                          bufs=2))
                pp1 = phctx.enter_context(tc.psum_pool(name="pp1", bufs=2))

                WqT = phW.tile([P, CO, C], F32, tag="WqT")
                WkT = phW.tile([P, CO, C], F32, tag="WkT")
                WvT = phW.tile([P, CO, C], F32, tag="WvT")
                WoT = phW.tile([P, CO, C], F32, tag="WoT")
                for WT, wsrc in (
                    (WqT, wqt_d), (WkT, wkt_d), (WvT, wvt_d), (WoT, wot_d)
                ):
                    # ACT-queue issue: SP stays dedicated to the x stream
                    nc.scalar.dma_start(
                        out=WT,
                        in_=wsrc[:].rearrange("(cio cp) co -> cp cio co",
                                              cp=P),
                    )
                # ---------- Phase A: stats stream + x -> fp8 ----------
                # x arrives in 4 DMAs of 1024 positions; the first two land in
                # the resident xloc (local query half, reused as the residual)
                stats = ph1.tile([P, CO, NXC, 6], F32, tag="stats")
                for ci in range(4):
                    csl = slice(ci * 1024, (ci + 1) * 1024)
                    if ci < 2:
                        xc = xloc[:, :, ci * 1024 : (ci + 1) * 1024]
                    else:
                        xc = xchunk.tile([P, CO, 1024], F32, tag="xc",
                                         name=f"xcs{ci}")
                    nc.sync.dma_start(out=xc, in_=xf_t[:, :, csl])
                    for coo in range(CO):
                        for hf in range(2):
                            nc.vector.bn_stats(
                                out=stats[:, coo, 2 * ci + hf, :],
                                in_=xc[:, coo, hf * 512 : (hf + 1) * 512],
                            )
                    nc.gpsimd.tensor_copy(out=x8[:, :, csl], in_=xc)
                mv = ph1.tile([P, CO, 2], F32, tag="mv")
                for coo in range(CO):
                    nc.vector.bn_aggr(out=mv[:, coo, :],
                                      in_=stats[:, coo, :, :])
                # T_in cols 0:4 per-channel mean, 4:8 per-channel E[x^2]
                T_in = ph1.tile([P, 8], F32, tag="T_in")
                nc.vector.tensor_copy(T_in[:, 0:CO], mv[:, :, 0])
                nc.vector.tensor_tensor(
                    out=T_in[:, CO : 2 * CO], in0=mv[:, :, 0], in1=mv[:, :, 0],
                    op=mybir.AluOpType.mult,
                )
                nc.vector.tensor_tensor(
                    out=T_in[:, CO : 2 * CO], in0=T_in[:, CO : 2 * CO],
                    in1=mv[:, :, 1], op=mybir.AluOpType.add,
                )
                tps = pp1.tile([8, P], F32, tag="wtp")
                nc.tensor.transpose(tps, T_in, ident)
                T_sb = ph1.tile([8, P], F32, tag="T_sb")
                nc.vector.tensor_copy(T_sb, tps)
                # G rows 0:4 = group-sums of channel means, 4:8 of E[x^2]
                G = ph1.tile([8, G_PER_CO], F32, tag="G")
                nc.vector.reduce_sum(
                    out=G, in_=T_sb.rearrange("p (g s) -> p g s", s=GS),
                    axis=mybir.AxisListType.X,
                )
                # broadcast group values along channels, then ONE transpose
                # back to channel layout — no cross-partition DMA bounce
                B2 = ph1.tile([8, P], F32, tag="B2")
                bc = bass.AP(
                    tensor=G.tensor, offset=G.offset,
                    ap=[G.ap[0], G.ap[1], [0, GS]],
                )
                nc.vector.tensor_copy(
                    B2.rearrange("p (g s) -> p g s", s=GS), bc
                )
                tc_ps = pp1.tile([P, 8], F32, tag="wtp", name="tc_ps")
                nc.tensor.transpose(tc_ps, B2, ident[0:8, 0:8])
                Tc = ph1.tile([P, 8], F32, tag="Tc")
                nc.vector.tensor_copy(Tc, tc_ps)
                Msum = Tc[:, 0:CO]       # per-group sum of means
                Ssum = Tc[:, CO : 2 * CO]  # per-group sum of E[x^2]
                # var*GS^2 = Ssum*GS - Msum^2 ;  rstd = GS/sqrt(var*GS^2 +
                # eps*GS^2) = exp(-0.5*ln(.) + ln(GS))
                M2 = ph1.tile([P, CO], F32, tag="M2")
                nc.vector.tensor_tensor(
                    out=M2, in0=Msum, in1=Msum, op=mybir.AluOpType.mult
                )
                Vg = ph1.tile([P, CO], F32, tag="Vg")
                nc.vector.scalar_tensor_tensor(
                    out=Vg, in0=Ssum, scalar=float(GS), in1=M2,
                    op0=mybir.AluOpType.mult, op1=mybir.AluOpType.subtract,
                )
                rstd_ch = ph1.tile([P, CO], F32, tag="rstd_ch")
                nc.scalar.activation(
                    out=rstd_ch, in_=Vg,
                    func=mybir.ActivationFunctionType.Ln,
                    bias=epsg_t, scale=1.0,
                )
                nc.scalar.activation(
                    out=rstd_ch, in_=rstd_ch, func=Exp, scale=-0.5,
                    bias=lngs_t,
                )
                a_ch = consts.tile([P, CO], F32, tag="a_ch")
                b_ch = consts.tile([P, CO], F32, tag="b_ch")
                nc.vector.tensor_tensor(
                    out=a_ch, in0=gamma_sb, in1=rstd_ch,
                    op=mybir.AluOpType.mult,
                )
                # b = beta - (Msum/GS) * a
                nc.vector.tensor_tensor(
                    out=b_ch, in0=Msum, in1=a_ch, op=mybir.AluOpType.mult
                )
                nc.vector.scalar_tensor_tensor(
                    out=b_ch, in0=b_ch, scalar=-1.0 / GS, in1=beta_sb,
                    op0=mybir.AluOpType.mult, op1=mybir.AluOpType.add,
                )

                # ---------- Phase B: bias folds + weight scale/cast ----------
                # b'_w = b_w + W b_ch in channel layout via N=1 f32 matmuls
                def fold_bias(WT, rhs_col_src, base_vec, bdst, nm):
                    for coo in range(CO):
                        pb = pp1.tile([P, 1], F32, tag="pb",
                                      name=f"pb_{nm}{coo}")
                        for cio in range(CO):
                            nc.tensor.matmul(
                                pb,
                                lhsT=WT[:, cio, coo * P : (coo + 1) * P],
                                rhs=rhs_col_src[:, cio : cio + 1],
                                start=(cio == 0), stop=(cio == CO - 1),
                            )
                        nc.vector.tensor_tensor(
                            out=bdst[:, coo : coo + 1], in0=pb,
                            in1=base_vec[:, coo : coo + 1],
                            op=mybir.AluOpType.add,
                        )

                fold_bias(WqT, b_ch, bq_ch, bqf, "q")
                # K bias dropped: it only adds a per-query constant to the
                # logits, which softmax over keys cancels exactly
                fold_bias(WvT, b_ch, bv_ch, bvf, "v")
                # V bias folds through the output projection:
                # bo' = bo + Wo b'v
                fold_bias(WoT, bvf, bo_ch, bof, "o")

                # W'8 = fp8(W * a) — per-input-channel scale; split engines
                for cio in range(CO):
                    nc.vector.tensor_scalar_mul(
                        Wq8[:, cio, :], WqT[:, cio, :], a_ch[:, cio : cio + 1]
                    )
                    nc.scalar.activation(
                        out=Wk8[:, cio, :], in_=WkT[:, cio, :], func=Identity,
                        scale=a_ch[:, cio : cio + 1], bias=zero_b,
                    )
                    nc.gpsimd.tensor_scalar_mul(
                        Wv8[:, cio, :], WvT[:, cio, :], a_ch[:, cio : cio + 1]
                    )
                nc.vector.tensor_copy(out=Wo8, in_=WoT)

                # PSUM->SBUF fp8 cast with per-partition bias, engine-rotated
                # GPSIMD cannot read PSUM (walrus birverifier) — PSUM->SBUF
                # casts run on DVE/ACT only
                def cast_bias(dst, src, bias_ap, eng):
                    if eng == 0:
                        nc.vector.tensor_scalar(
                            out=dst, in0=src, scalar1=bias_ap, scalar2=None,
                            op0=mybir.AluOpType.add,
                        )
                    else:
                        nc.scalar.activation(
                            out=dst, in_=src, func=Identity, bias=bias_ap,
                            scale=1.0,
                        )

                # ---------- Phase C: Q projection (fp8 DR) ----------
                nce = 0
                for coo in range(CO):
                    for ib in range(NIB):
                        isl = slice(ib * IB, (ib + 1) * IB)
                        pq = po.tile([P, IB], F32, tag="ops",
                                     name=f"pq{ib}_{coo}")
                        for q in range(2):
                            nc.tensor.matmul(
                                pq,
                                lhsT=Wq8[:, 2 * q : 2 * q + 2,
                                         coo * P : (coo + 1) * P],
                                rhs=x8[:, 2 * q : 2 * q + 2, isl],
                                start=(q == 0), stop=(q == 1), perf_mode=DR,
                            )
                        cast_bias(Q8[:, coo, isl], pq,
                                  bqf[:, coo : coo + 1], nce % 2)
                        nce += 1

                # ---------- Phase D: K and V^T, interleaved per 512-col
                # block so attention consumption order matches production ----
                for s in range(NXC):
                    jsl = slice(s * 512, (s + 1) * 512)
                    for coo in range(CO):
                        pk = po.tile([P, 512], F32, tag="ops",
                                     name=f"pk{s}_{coo}")
                        for q in range(2):
                            nc.tensor.matmul(
                                pk,
                                lhsT=Wk8[:, 2 * q : 2 * q + 2,
                                         coo * P : (coo + 1) * P],
                                rhs=x8[:, 2 * q : 2 * q + 2, jsl],
                                start=(q == 0), stop=(q == 1), perf_mode=DR,
                            )
                        # no bias (dropped: softmax-invariant): pure cast
                        dstk = K8[:, coo, jsl]
                        eng = nce % 2
                        nce += 1
                        if eng == 0:
                            nc.vector.tensor_copy(out=dstk, in_=pk)
                        else:
                            nc.scalar.activation(
                                out=dstk, in_=pk, func=Identity, scale=1.0,
                                bias=zero_b,
                            )
                    for jb in (2 * s, 2 * s + 1):
                        for jh in range(2):
                            jpos = jb * JB + jh * P
                            pv = po.tile([P, C], F32, tag="ops",
                                         name=f"pv{jb}_{jh}")
                            for q in range(2):
                                nc.tensor.matmul(
                                    pv,
                                    lhsT=x8[:, 2 * q : 2 * q + 2,
                                            jpos : jpos + P],
                                    rhs=Wv8[:, 2 * q : 2 * q + 2, :],
                                    start=(q == 0), stop=(q == 1),
                                    perf_mode=DR,
                                )
                            dstv = VT8[:, jb, jh, :]
                            eng = nce % 2
                            nce += 1
                            if eng == 0:
                                nc.vector.tensor_copy(out=dstv, in_=pv)
                            else:
                                nc.scalar.activation(
                                    out=dstv, in_=pv, func=Identity,
                                    scale=1.0, bias=zero_b,
                                )

            # ---------- Phase E: attention + output ----------
            # Late normalization: O is cast to fp8 unnormalized (pre-scaled
            # by 1/128 to stay in fp8 range) the moment the j-loop ends,
            # releasing the 4 O PSUM banks immediately; the output projection
            # runs on unnormalized O and the 128/denom factor is applied in
            # the final DVE op. The out-projection PE work is interleaved
            # into the NEXT query block's pipeline slots so the in-order PE
            # queue never stalls on the output tail.
            OSC = 1.0 / 128.0
            with (
                tc.tile_pool(name="att", bufs=2) as att,
                tc.tile_pool(name="esb", bufs=4) as esb,
                tc.psum_pool(name="pspt", bufs=2) as pspt,
                tc.psum_pool(name="pdns", bufs=1) as pdns,
                tc.psum_pool(name="pfb", bufs=1) as pfb,
            ):
                tail = []  # pending PE op-groups from the previous ib
                carry = [None]  # last O/denominator group, carried across ibs

                def gp_bias_res_add(t, coo, xres):
                    # t += bo' (free-broadcast AP) ; t += x  — SBUF-only ops
                    # on the otherwise idle GPSIMD engine
                    bof_b = bass.AP(
                        tensor=bof.tensor, offset=bof.offset + coo,
                        ap=[bof.ap[0], [0, IB]],
                    )
                    nc.gpsimd.tensor_tensor(
                        out=t, in0=t, in1=bof_b, op=mybir.AluOpType.add,
                    )
                    nc.gpsimd.tensor_tensor(
                        out=t, in0=t, in1=xres[:, coo, :],
                        op=mybir.AluOpType.add,
                    )


                def emit_out_tail(ib, O8, bcast_sb, ysb, xres, isl):
                    def do_bcast(recip=None, _ib=ib):
                        bcast_ps = pfb.tile([P, IB], F32, tag="fps",
                                            name=f"bc{_ib}")
                        nc.tensor.matmul(
                            bcast_ps, lhsT=ones_row, rhs=recip, start=True,
                            stop=True,
                        )
                        # 128/denom: undo the 1/128 O pre-scale
                        nc.vector.tensor_scalar(
                            out=bcast_sb, in0=bcast_ps, scalar1=128.0,
                            scalar2=None, op0=mybir.AluOpType.mult,
                        )

                    def do_coo(coo, _ib=ib):
                        fps = pfb.tile([P, IB], F32, tag="fps",
                                       name=f"fps{_ib}_{coo}")
                        for q in range(2):
                            nc.tensor.matmul(
                                fps,
                                lhsT=Wo8[:, 2 * q : 2 * q + 2,
                                         coo * P : (coo + 1) * P],
                                rhs=O8[:, 2 * q : 2 * q + 2, :],
                                start=(q == 0), stop=(q == 1), perf_mode=DR,
                            )
                        t = ysb[:, coo, :]
                        nc.vector.tensor_tensor(
                            out=t, in0=fps, in1=bcast_sb,
                            op=mybir.AluOpType.mult,
                        )
                        if coo % 2 == 0:
                            nc.vector.scalar_tensor_tensor(
                                out=t, in0=t, scalar=bof[:, coo : coo + 1],
                                in1=xres[:, coo, :],
                                op0=mybir.AluOpType.add,
                                op1=mybir.AluOpType.add,
                            )
                        else:
                            nc.gpsimd.scalar_tensor_tensor(
                                out=t, in0=t, scalar=bof[:, coo : coo + 1],
                                in1=xres[:, coo, :],
                                op0=mybir.AluOpType.add,
                                op1=mybir.AluOpType.add,
                            )

                    def do_store(_isl=isl, _ysb=ysb):
                        nc.sync.dma_start(out=y_t[:, :, _isl], in_=_ysb)

                    return [do_bcast] + [
                        (lambda c: lambda: do_coo(c))(c) for c in range(CO)
                    ] + [do_store]

                for ib in range(NIB):
                    isl = slice(ib * IB, (ib + 1) * IB)
                    xres = xloc[:, :, isl]
                    ops = [
                        po.tile([P, IB], F32, tag="ops", name=f"ops{ib}_{i}")
                        for i in range(CO)
                    ]
                    dns = pdns.tile([16, IB], F32, tag="dns")

                    # Software-pipelined emission: PE executes in program
                    # order, so scores(jb) are emitted BEFORE the previous
                    # block's O/denominator matmuls — scores(jb) can then run
                    # while ACT is still exponentiating block jb-1, keeping
                    # ACT back-to-back (the loop's bottleneck).
                    def emit_scores(jb):
                        # two single-bank score tiles (separate pool slots so
                        # the next block's h0 scores only wait on THIS h0's
                        # exp read, not both halves)
                        e8 = esb.tile([P, 2, IB], FP8, tag="e8",
                                      name=f"e8_{ib}_{jb}")
                        for h in range(2):
                            spt = pspt.tile([P, IB], F32, tag="spt",
                                            name=f"spt{ib}_{jb}_{h}")
                            for q in range(2):
                                nc.tensor.matmul(
                                    spt,
                                    lhsT=K8[:, 2 * q : 2 * q + 2,
                                            jb * JB + h * P : jb * JB
                                            + (h + 1) * P],
                                    rhs=Q8[:, 2 * q : 2 * q + 2, isl],
                                    start=(q == 0), stop=(q == 1),
                                    perf_mode=DR,
                                )
                            nc.scalar.activation(
                                out=e8[:, h, :], in_=spt, func=Exp,
                                scale=SCALE, bias=mshift,
                            )
                        return e8

                    def emit_ov(jb, e8, _ops=ops, _dns=dns):
                        for cio in range(CO):
                            nc.tensor.matmul(
                                _ops[cio],
                                lhsT=VT8[:, jb, :, cio * P : (cio + 1) * P],
                                rhs=e8,
                                start=(jb == 0), stop=(jb == NJB - 1),
                                perf_mode=DR,
                            )
                        nc.tensor.matmul(
                            _dns, lhsT=ones8, rhs=e8,
                            start=(jb == 0), stop=(jb == NJB - 1),
                            perf_mode=DR,
                        )

                    for jb in range(NJB):
                        e8 = emit_scores(jb)
                        if carry[0] is not None:
                            f, args = carry[0]
                            f(*args)
                            carry[0] = None
                        if tail:
                            tail.pop(0)()
                        carry[0] = (emit_ov, (jb, e8))
                    if ib == NIB - 1:
                        f, args = carry[0]
                        f(*args)
                        carry[0] = None
                    while tail and ib == NIB - 1:
                        tail.pop(0)()

                    last = ib == NIB - 1
                    O8 = att.tile([P, CO, IB], FP8, tag="O8")
                    recip = att.tile([1, IB], F32R, tag="recip")

                    def emit_drains(_ops=ops, _dns=dns, _O8=O8, _recip=recip,
                                    _last=last):
                        # drain O to SBUF fp8 (frees the 4 O banks);
                        # on the last block use both engines
                        for cio in range(CO):
                            if _last and cio % 2 == 0:
                                nc.scalar.activation(
                                    out=_O8[:, cio, :], in_=_ops[cio],
                                    func=Identity, scale=OSC, bias=zero_b,
                                )
                            else:
                                nc.vector.tensor_scalar(
                                    out=_O8[:, cio, :], in0=_ops[cio],
                                    scalar1=OSC, scalar2=None,
                                    op0=mybir.AluOpType.mult,
                                )
                        with nc.allow_low_precision(
                                reason="f32r holds fp32 bits"):
                            nc.vector.reciprocal(out=_recip,
                                                 in_=_dns[0:1, :])

                    bcast_sb = att.tile([P, IB], F32, tag="bcast_sb")
                    ysb = att.tile([P, CO, IB], F32, tag="ysb")
                    if not last:
                        items = emit_out_tail(ib, O8, bcast_sb, ysb, xres,
                                              isl)
                        items[0] = (lambda f, r: lambda: f(r))(items[0],
                                                              recip)
                        tail = [emit_drains] + items
                    else:
                        emit_drains()
                        # final block: nothing left to overlap with — use the
                        # freed O banks for a parallel out-projection and
                        # stream y out in half-blocks
                        bcast_ps = pfb.tile([P, IB], F32, tag="fps",
                                            name="bc_last")
                        nc.tensor.matmul(
                            bcast_ps, lhsT=ones_row, rhs=recip, start=True,
                            stop=True,
                        )
                        nc.vector.tensor_scalar(
                            out=bcast_sb, in0=bcast_ps, scalar1=128.0,
                            scalar2=None, op0=mybir.AluOpType.mult,
                        )
                        for coo in range(CO):
                            fps = po.tile([P, IB], F32, tag="ops",
                                          name=f"fpsL_{coo}")
                            for q in range(2):
                                nc.tensor.matmul(
                                    fps,
                                    lhsT=Wo8[:, 2 * q : 2 * q + 2,
                                             coo * P : (coo + 1) * P],
                                    rhs=O8[:, 2 * q : 2 * q + 2, :],
                                    start=(q == 0), stop=(q == 1),
                                    perf_mode=DR,
                                )
                            t = ysb[:, coo, :]
                            nc.vector.tensor_tensor(
                                out=t, in0=fps, in1=bcast_sb,
                                op=mybir.AluOpType.mult,
                            )
                            if coo % 2 == 0:
                                nc.vector.scalar_tensor_tensor(
                                    out=t, in0=t,
                                    scalar=bof[:, coo : coo + 1],
                                    in1=xres[:, coo, :],
                                    op0=mybir.AluOpType.add,
                                    op1=mybir.AluOpType.add,
                                )
                            else:
                                nc.gpsimd.scalar_tensor_tensor(
                                    out=t, in0=t,
                                    scalar=bof[:, coo : coo + 1],
                                    in1=xres[:, coo, :],
                                    op0=mybir.AluOpType.add,
                                    op1=mybir.AluOpType.add,
                                )
                            if coo % 2 == 1:
                                nc.sync.dma_start(
                                    out=y_t[:, coo - 1 : coo + 1, isl],
                                    in_=ysb[:, coo - 1 : coo + 1, :],
                                )
                while tail:
                    tail.pop(0)()

    nc.compile()
    return nc


def get_program(reps: int = 1):
    key = f"nc{reps}"
    if key not in _cached:
        _cached[key] = build_program(reps)
    return _cached[key]


def make_in_maps(inputs):
    x = np.asarray(inputs["x"], np.float32).reshape(B, C, HW)
    common = {
        k: np.ascontiguousarray(np.asarray(inputs[k], np.float32))
        for k in ("bq", "bk", "bv", "bo", "gamma", "beta")
    }
    for k in ("wq", "wk", "wv", "wo"):
        common[k + "t"] = np.ascontiguousarray(np.asarray(inputs[k], np.float32).T)
    in_maps = []
    for core in range(NCORES):
        b, h = core // 2, core % 2
        loc = x[b][:, h * L : (h + 1) * L]
        oth = x[b][:, (1 - h) * L : (2 - h) * L]
        xf_rot = np.ascontiguousarray(np.concatenate([loc, oth], axis=1))
        m = dict(common)
        m["xf"] = xf_rot
        in_maps.append(m)
    return in_maps


def kernel(**inputs) -> np.ndarray:
    from concourse.bass_utils import run_bass_kernel_spmd

    nc = get_program()
    in_maps = make_in_maps(inputs)
    res = run_bass_kernel_spmd(nc, in_maps, list(range(NCORES)))
    out = np.empty((B, C, HW), np.float32)
    for core in range(NCORES):
        b, h = core // 2, core % 2
        out[b][:, h * L : (h + 1) * L] = res.results[core]["y"]
    return out.reshape(B, C, 64, 64)
